# revision 1
# baseline (speedup 1.0000x reference)
"""Trainium2 Bass kernel for BrainInspiredEmotionGraph (2-layer RGCN, 17 nodes,
8 relations, d=2048) running SPMD on 8 NeuronCores.

Math: layer(x) = sum_r A_r @ x @ W_r + x @ root + bias, where A_r is the
[17,17] per-relation mean-aggregation matrix built from the edge list.
h1 = relu(layer1(h)); out = layer2(h1), h = node_emb with signal rows patched.

Sharding: output-column sharding. Core c owns columns [c*256,(c+1)*256) of
both layers' outputs, so it reads W1[:, :, chunk], root1[:, chunk], W2[...],
root2[...] (37.75 MB/core — the memory-roofline term). The tiny [17,2048]
activation h1 is exchanged transposed via an 8-core AllGather.

Layer 1 uses host-premixed lhsT ((A_r h)^T per relation + h^T for the root),
so it is a single 145-matmul accumulation chain into one PSUM tile.
Layer 2 computes Y_r = h1 @ W2_r_chunk on-device, then out += A_r @ Y_r
(one extra [17,17]x[17,256] matmul per relation); root2/bias accumulate
directly. Weight slabs stream as contiguous 2 MB DMAs with a permuted
contraction order (partition p holds K-rows {16p+j}) so both the DMA and
the AllGather output need zero on-device rearrangement.
"""
import sys

if '/opt/trn_rl_repo' not in sys.path:
    sys.path.insert(0, '/opt/trn_rl_repo')

import numpy as np
from concourse import bacc, tile, mybir, bass_utils

N_NODES = 17
N_REL = 8
D = 2048
N_CORES = 8
CH = D // N_CORES          # 256 output columns per core
KT = 128                    # contraction rows per matmul
JT = D // KT                # 16 k-tiles per slab
NSLAB = 2 * (N_REL + 1)     # 18 weight slabs (9 per layer)
F32 = mybir.dt.float32

_compiled = None


def _build():
    nc = bacc.Bacc("TRN2", target_bir_lowering=False, debug=False,
                   num_devices=N_CORES)
    wcat = nc.dram_tensor("wcat", [NSLAB, KT, JT * CH], F32,
                          kind="ExternalInput").ap()
    x1t = nc.dram_tensor("x1t", [KT, 9 * JT * N_NODES], F32,
                         kind="ExternalInput").ap()
    at = nc.dram_tensor("at", [N_NODES, N_REL * N_NODES], F32,
                        kind="ExternalInput").ap()
    b1 = nc.dram_tensor("b1", [1, CH], F32, kind="ExternalInput").ap()
    b2 = nc.dram_tensor("b2", [1, CH], F32, kind="ExternalInput").ap()
    ident = nc.dram_tensor("ident", [N_NODES, N_NODES], F32,
                           kind="ExternalInput").ap()
    ones = nc.dram_tensor("ones", [1, N_NODES], F32,
                          kind="ExternalInput").ap()
    out = nc.dram_tensor("out", [N_NODES, CH], F32,
                         kind="ExternalOutput").ap()

    with tile.TileContext(nc) as tc:
        with tc.tile_pool(name="const", bufs=1) as constp, \
             tc.tile_pool(name="wpool", bufs=8) as wpool, \
             tc.tile_pool(name="spool", bufs=2) as spool, \
             tc.tile_pool(name="opsum", bufs=1, space="PSUM") as opsum, \
             tc.tile_pool(name="ypsum", bufs=2, space="PSUM") as ypsum, \
             tc.tile_pool(name="tpsum", bufs=2, space="PSUM") as tpsum, \
             tc.tile_pool(name="dram", bufs=1, space="DRAM") as dram:

            x1t_sb = constp.tile([KT, 9 * JT * N_NODES], F32)
            nc.scalar.dma_start(out=x1t_sb, in_=x1t)
            at_sb = constp.tile([N_NODES, N_REL * N_NODES], F32)
            nc.scalar.dma_start(out=at_sb, in_=at)
            b1_sb = constp.tile([1, CH], F32)
            nc.scalar.dma_start(out=b1_sb, in_=b1)
            b2_sb = constp.tile([1, CH], F32)
            nc.scalar.dma_start(out=b2_sb, in_=b2)
            id_sb = constp.tile([N_NODES, N_NODES], F32)
            nc.scalar.dma_start(out=id_sb, in_=ident)
            ones_sb = constp.tile([1, N_NODES], F32)
            nc.scalar.dma_start(out=ones_sb, in_=ones)

            # ---------------- layer 1 ----------------
            out1 = opsum.tile([N_NODES, CH], F32, name="out1")
            nc.tensor.matmul(out1, lhsT=ones_sb, rhs=b1_sb,
                             start=True, stop=False)
            for s in range(9):
                w = wpool.tile([KT, JT * CH], F32, name="wslab", tag="wslab")
                nc.sync.dma_start(out=w, in_=wcat[s])
                for j in range(JT):
                    k = s * JT + j
                    nc.tensor.matmul(
                        out1,
                        lhsT=x1t_sb[:, k * N_NODES:(k + 1) * N_NODES],
                        rhs=w[:, j * CH:(j + 1) * CH],
                        start=False, stop=(s == 8 and j == JT - 1))
            h1 = spool.tile([N_NODES, CH], F32, name="h1")
            nc.scalar.activation(h1, out1, mybir.ActivationFunctionType.Relu)

            # transpose h1 chunk -> [256,17], AllGather -> h1T [2048,17]
            agin = dram.tile([2 * KT, N_NODES], F32, name="agin")
            for half in range(2):
                tp = tpsum.tile([KT, N_NODES], F32, name="tp")
                nc.tensor.transpose(tp, h1[:, half * KT:(half + 1) * KT],
                                    id_sb)
                tps = spool.tile([KT, N_NODES], F32, name="tps")
                nc.vector.tensor_copy(tps, tp)
                nc.scalar.dma_start(out=agin[half * KT:(half + 1) * KT, :],
                                    in_=tps)
            agout = dram.tile([KT, JT * N_NODES], F32, name="agout",
                              addr_space="Shared")
            nc.gpsimd.collective_compute(
                "AllGather", mybir.AluOpType.bypass,
                replica_groups=[list(range(N_CORES))],
                ins=[agin.opt()], outs=[agout.opt()])
            h1t_sb = spool.tile([KT, JT * N_NODES], F32, name="h1t_sb")
            nc.scalar.dma_start(out=h1t_sb, in_=agout)

            # ---------------- layer 2 ----------------
            out2 = opsum.tile([N_NODES, CH], F32, name="out2")
            nc.tensor.matmul(out2, lhsT=ones_sb, rhs=b2_sb,
                             start=True, stop=False, skip_group_check=True)
            for s in range(9):
                w = wpool.tile([KT, JT * CH], F32, name="wslab", tag="wslab")
                nc.sync.dma_start(out=w, in_=wcat[9 + s])
                if s < N_REL:
                    y = ypsum.tile([N_NODES, CH], F32, name="y")
                    for j in range(JT):
                        nc.tensor.matmul(
                            y,
                            lhsT=h1t_sb[:, j * N_NODES:(j + 1) * N_NODES],
                            rhs=w[:, j * CH:(j + 1) * CH],
                            start=(j == 0), stop=(j == JT - 1),
                            skip_group_check=True)
                    ysb = spool.tile([N_NODES, CH], F32, name="ysb")
                    nc.vector.tensor_copy(ysb, y)
                    nc.tensor.matmul(
                        out2, lhsT=at_sb[:, s * N_NODES:(s + 1) * N_NODES],
                        rhs=ysb, start=False, stop=False,
                        skip_group_check=True)
                else:  # root2
                    for j in range(JT):
                        nc.tensor.matmul(
                            out2,
                            lhsT=h1t_sb[:, j * N_NODES:(j + 1) * N_NODES],
                            rhs=w[:, j * CH:(j + 1) * CH],
                            start=False, stop=(j == JT - 1),
                            skip_group_check=True)
            osb = spool.tile([N_NODES, CH], F32, name="osb")
            nc.vector.tensor_copy(osb, out2)
            nc.scalar.dma_start(out=out, in_=osb)

    nc.compile()
    return nc


def _prep_inputs(inputs):
    """Host-side prep: build A matrices, premixed layer-1 lhsT, and the
    per-core permuted weight stacks."""
    h = np.array(inputs['node_emb'], dtype=np.float32, copy=True)
    sf = np.asarray(inputs['signal_features'], dtype=np.float32)
    h[:sf.shape[0]] = sf
    src = np.asarray(inputs['edge_index'])[0].astype(np.int64)
    dst = np.asarray(inputs['edge_index'])[1].astype(np.int64)
    et = np.asarray(inputs['edge_type']).astype(np.int64)

    A = np.zeros((N_REL, N_NODES, N_NODES), np.float32)
    cnt = np.zeros((N_REL, N_NODES), np.float32)
    np.add.at(cnt, (et, dst), 1.0)
    np.add.at(A, (et, dst, src), 1.0)
    A /= np.maximum(cnt, 1.0)[:, :, None]

    # layer-1 lhsT: 9 slabs of (A_r h)^T (+ h^T for root), K-permuted so
    # partition p holds rows {16p+j}: [128, 9*16*17]
    Z = np.concatenate([np.einsum('rij,jd->rid', A, h,
                                  dtype=np.float32).astype(np.float32),
                        h[None]], axis=0)           # [9,17,2048]
    x1t = (Z.transpose(0, 2, 1)                      # [9,2048,17]
            .reshape(9, KT, JT, N_NODES)
            .transpose(1, 0, 2, 3)
            .reshape(KT, 9 * JT * N_NODES)).astype(np.float32).copy()

    # A_r^T stacked along columns: at[n, r*17+m] = A[r][m, n]
    at = (A.transpose(0, 2, 1).transpose(1, 0, 2)
           .reshape(N_NODES, N_REL * N_NODES)).astype(np.float32).copy()

    W1 = np.asarray(inputs['W1'], dtype=np.float32)
    W2 = np.asarray(inputs['W2'], dtype=np.float32)
    r1 = np.asarray(inputs['root1'], dtype=np.float32)
    r2 = np.asarray(inputs['root2'], dtype=np.float32)
    bias1 = np.asarray(inputs['bias1'], dtype=np.float32)
    bias2 = np.asarray(inputs['bias2'], dtype=np.float32)
    Wfull = np.concatenate([W1, r1[None], W2, r2[None]], axis=0)  # [18,2048,2048]

    ident = np.eye(N_NODES, dtype=np.float32)
    ones = np.ones((1, N_NODES), np.float32)

    in_maps = []
    for c in range(N_CORES):
        cols = slice(c * CH, (c + 1) * CH)
        wc = (Wfull[:, :, cols]
              .reshape(NSLAB, KT, JT, CH)
              .reshape(NSLAB, KT, JT * CH)).astype(np.float32).copy()
        in_maps.append({
            'wcat': wc,
            'x1t': x1t,
            'at': at,
            'b1': np.ascontiguousarray(bias1[cols][None, :]),
            'b2': np.ascontiguousarray(bias2[cols][None, :]),
            'ident': ident,
            'ones': ones,
        })
    return in_maps


def get_compiled():
    global _compiled
    if _compiled is None:
        _compiled = _build()
    return _compiled


def run(inputs, trace=False):
    nc = get_compiled()
    in_maps = _prep_inputs(inputs)
    res = bass_utils.run_bass_kernel_spmd(
        nc, in_maps, core_ids=list(range(N_CORES)), trace=trace)
    outp = np.concatenate(
        [np.asarray(res.results[c]['out']) for c in range(N_CORES)], axis=1)
    return outp.astype(np.float32), res


def kernel(**inputs):
    outp, _ = run(inputs, trace=False)
    return outp


# revision 2
# speedup vs baseline: 1.2643x; 1.2643x over previous
"""Trainium2 Bass kernel for BrainInspiredEmotionGraph (2-layer RGCN, 17 nodes,
8 relations, d=2048) running SPMD on 8 NeuronCores.

Math: layer(x) = sum_r A_r @ x @ W_r + x @ root + bias, where A_r is the
[17,17] per-relation mean-aggregation matrix built from the edge list.
h1 = relu(layer1(h)); out = layer2(h1), h = node_emb with signal rows patched.

Sharding (fully collective-free):
- Layer 1: output-column sharding. Core c computes h1[:, c*256:(c+1)*256]
  from W1[:, :, chunk] + root1[:, chunk] (host-premixed lhsT: (A_r h)^T per
  relation + h^T for the root, so it is one 145-matmul PSUM accumulation).
- Layer 2: hidden-dim contraction sharding. Core c computes the partial
  P_c = sum_r (A_r h1[:, chunk]) @ W2_r[chunk, :] + h1[:, chunk] @ root2[chunk, :]
  over the h1 columns it already owns — no inter-core exchange. The host
  sums the 8 [17, 2048] partials and adds bias2.
Per-core HBM traffic is the roofline term: 37.75 MB of fp32 weights,
streamed as contiguous 2 MB slabs (16 KB per partition per DMA).
"""
import sys

if '/opt/trn_rl_repo' not in sys.path:
    sys.path.insert(0, '/opt/trn_rl_repo')

import numpy as np
from concourse import bacc, tile, mybir, bass_utils

N_NODES = 17
N_REL = 8
D = 2048
N_CORES = 8
CH = D // N_CORES          # 256 columns of h1 owned per core
KT = 128                    # contraction rows per matmul
JT = D // KT                # 16 k-tiles per layer-1 slab
NSTRIP = 4                  # layer-2 output strips of 512 columns
F32 = mybir.dt.float32

_compiled = None


def _build():
    nc = bacc.Bacc("TRN2", target_bir_lowering=False, debug=False,
                   num_devices=N_CORES)
    # layer-1 weights: 9 slabs [128, 16*256], K-permuted (partition p holds
    # rows {16p+j}); layer-2 weights: 9 slabs [128, 2*2048] (partition p
    # holds rows p and 128+p of the core's 256-row band).
    w1 = nc.dram_tensor("w1", [9, KT, JT * CH], F32,
                        kind="ExternalInput").ap()
    w2 = nc.dram_tensor("w2", [9, KT, 2 * D], F32,
                        kind="ExternalInput").ap()
    x1t = nc.dram_tensor("x1t", [KT, 9 * JT * N_NODES], F32,
                         kind="ExternalInput").ap()
    at = nc.dram_tensor("at", [N_NODES, N_REL * N_NODES], F32,
                        kind="ExternalInput").ap()
    b1 = nc.dram_tensor("b1", [1, CH], F32, kind="ExternalInput").ap()
    ident = nc.dram_tensor("ident", [N_NODES, N_NODES], F32,
                           kind="ExternalInput").ap()
    ones = nc.dram_tensor("ones", [1, N_NODES], F32,
                          kind="ExternalInput").ap()
    out = nc.dram_tensor("out", [N_NODES, D], F32,
                         kind="ExternalOutput").ap()

    with tile.TileContext(nc) as tc:
        with tc.tile_pool(name="const", bufs=1) as constp, \
             tc.tile_pool(name="wpool", bufs=8) as wpool, \
             tc.tile_pool(name="spool", bufs=2) as spool, \
             tc.tile_pool(name="opsum", bufs=1, space="PSUM") as opsum, \
             tc.tile_pool(name="ppsum", bufs=2, space="PSUM") as ppsum:

            x1t_sb = constp.tile([KT, 9 * JT * N_NODES], F32)
            nc.scalar.dma_start(out=x1t_sb, in_=x1t)
            at_sb = constp.tile([N_NODES, N_REL * N_NODES], F32)
            nc.scalar.dma_start(out=at_sb, in_=at)
            b1_sb = constp.tile([1, CH], F32)
            nc.scalar.dma_start(out=b1_sb, in_=b1)
            id_sb = constp.tile([N_NODES, N_NODES], F32)
            nc.scalar.dma_start(out=id_sb, in_=ident)
            ones_sb = constp.tile([1, N_NODES], F32)
            nc.scalar.dma_start(out=ones_sb, in_=ones)

            # ---------------- layer 1 ----------------
            out1 = opsum.tile([N_NODES, CH], F32, name="out1")
            nc.tensor.matmul(out1, lhsT=ones_sb, rhs=b1_sb,
                             start=True, stop=False)
            for s in range(9):
                w = wpool.tile([KT, JT * CH], F32, name="wslab", tag="wslab")
                nc.sync.dma_start(out=w, in_=w1[s])
                for j in range(JT):
                    k = s * JT + j
                    nc.tensor.matmul(
                        out1,
                        lhsT=x1t_sb[:, k * N_NODES:(k + 1) * N_NODES],
                        rhs=w[:, j * CH:(j + 1) * CH],
                        start=False, stop=(s == 8 and j == JT - 1))
            h1 = spool.tile([N_NODES, CH], F32, name="h1")
            nc.scalar.activation(h1, out1, mybir.ActivationFunctionType.Relu)

            # layer-2 lhsT prep: (A_r h1_c)^T tiles for r<8, h1_c^T for root
            xt2_sb = spool.tile([KT, 18 * N_NODES], F32, name="xt2_sb")
            for s in range(9):
                rhs = (at_sb[:, s * N_NODES:(s + 1) * N_NODES]
                       if s < N_REL else id_sb)
                for kt in range(2):
                    pp = ppsum.tile([KT, N_NODES], F32, name="pp")
                    nc.tensor.matmul(pp, lhsT=h1[:, kt * KT:(kt + 1) * KT],
                                     rhs=rhs, start=True, stop=True)
                    i = s * 2 + kt
                    nc.vector.tensor_copy(
                        xt2_sb[:, i * N_NODES:(i + 1) * N_NODES], pp)

            # ---------------- layer 2 (partial over owned h1 columns) -----
            out2 = [None] * NSTRIP
            for n in range(NSTRIP):
                t = opsum.tile([N_NODES, 512], F32, name=f"out2_{n}",
                               tag=f"out2_{n}")
                out2[n] = t
            for s in range(9):
                w = wpool.tile([KT, 2 * D], F32, name="wslab", tag="wslab")
                nc.sync.dma_start(out=w, in_=w2[s])
                for kt in range(2):
                    i = s * 2 + kt
                    lhsT = xt2_sb[:, i * N_NODES:(i + 1) * N_NODES]
                    for n in range(NSTRIP):
                        nc.tensor.matmul(
                            out2[n], lhsT=lhsT,
                            rhs=w[:, kt * D + n * 512: kt * D + (n + 1) * 512],
                            start=(s == 0 and kt == 0),
                            stop=(s == 8 and kt == 1),
                            skip_group_check=True)
            osb = spool.tile([N_NODES, D], F32, name="osb")
            for n in range(NSTRIP):
                nc.vector.tensor_copy(osb[:, n * 512:(n + 1) * 512], out2[n])
            nc.scalar.dma_start(out=out, in_=osb)

    nc.compile()
    return nc


def _prep_inputs(inputs):
    """Host-side prep: A matrices, premixed layer-1 lhsT, per-core weights."""
    h = np.array(inputs['node_emb'], dtype=np.float32, copy=True)
    sf = np.asarray(inputs['signal_features'], dtype=np.float32)
    h[:sf.shape[0]] = sf
    src = np.asarray(inputs['edge_index'])[0].astype(np.int64)
    dst = np.asarray(inputs['edge_index'])[1].astype(np.int64)
    et = np.asarray(inputs['edge_type']).astype(np.int64)

    A = np.zeros((N_REL, N_NODES, N_NODES), np.float32)
    cnt = np.zeros((N_REL, N_NODES), np.float32)
    np.add.at(cnt, (et, dst), 1.0)
    np.add.at(A, (et, dst, src), 1.0)
    A /= np.maximum(cnt, 1.0)[:, :, None]

    # layer-1 lhsT: 9 slabs of (A_r h)^T (+ h^T for root), K-permuted so
    # partition p holds rows {16p+j}: [128, 9*16*17]
    Z = np.concatenate([np.einsum('rij,jd->rid', A, h).astype(np.float32),
                        h[None]], axis=0)           # [9,17,2048]
    x1t = (Z.transpose(0, 2, 1)                      # [9,2048,17]
            .reshape(9, KT, JT, N_NODES)
            .transpose(1, 0, 2, 3)
            .reshape(KT, 9 * JT * N_NODES)).astype(np.float32).copy()

    # A_r^T stacked along columns: at[n, r*17+m] = A[r][m, n]
    at = (A.transpose(0, 2, 1).transpose(1, 0, 2)
           .reshape(N_NODES, N_REL * N_NODES)).astype(np.float32).copy()

    W1 = np.asarray(inputs['W1'], dtype=np.float32)
    W2 = np.asarray(inputs['W2'], dtype=np.float32)
    r1 = np.asarray(inputs['root1'], dtype=np.float32)
    r2 = np.asarray(inputs['root2'], dtype=np.float32)
    bias1 = np.asarray(inputs['bias1'], dtype=np.float32)
    W1full = np.concatenate([W1, r1[None]], axis=0)   # [9,2048,2048]
    W2full = np.concatenate([W2, r2[None]], axis=0)   # [9,2048,2048]

    ident = np.eye(N_NODES, dtype=np.float32)
    ones = np.ones((1, N_NODES), np.float32)

    in_maps = []
    for c in range(N_CORES):
        cols = slice(c * CH, (c + 1) * CH)
        w1c = (W1full[:, :, cols]
               .reshape(9, KT, JT, CH)
               .reshape(9, KT, JT * CH)).astype(np.float32).copy()
        w2c = (W2full[:, cols, :]
               .reshape(9, 2, KT, D)
               .transpose(0, 2, 1, 3)
               .reshape(9, KT, 2 * D)).astype(np.float32).copy()
        in_maps.append({
            'w1': w1c,
            'w2': w2c,
            'x1t': x1t,
            'at': at,
            'b1': np.ascontiguousarray(bias1[cols][None, :]),
            'ident': ident,
            'ones': ones,
        })
    return in_maps


def get_compiled():
    global _compiled
    if _compiled is None:
        _compiled = _build()
    return _compiled


def run(inputs, trace=False):
    nc = get_compiled()
    in_maps = _prep_inputs(inputs)
    res = bass_utils.run_bass_kernel_spmd(
        nc, in_maps, core_ids=list(range(N_CORES)), trace=trace)
    acc = np.zeros((N_NODES, D), np.float64)
    for c in range(N_CORES):
        acc += np.asarray(res.results[c]['out'], dtype=np.float64)
    acc += np.asarray(inputs['bias2'], dtype=np.float64)[None, :]
    return acc.astype(np.float32), res


def kernel(**inputs):
    outp, _ = run(inputs, trace=False)
    return outp


# revision 6
# speedup vs baseline: 1.4663x; 1.1597x over previous
"""Trainium2 Bass kernel for BrainInspiredEmotionGraph (2-layer RGCN, 17 nodes,
8 relations, d=2048) running SPMD on 8 NeuronCores.

Math: layer(x) = sum_r A_r @ x @ W_r + x @ root + bias, where A_r is the
[17,17] per-relation mean-aggregation matrix built from the edge list.
h1 = relu(layer1(h)); out = layer2(h1), h = node_emb with signal rows patched.

Sharding (fully collective-free):
- Layer 1: output-column sharding. Core c computes h1[:, c*256:(c+1)*256]
  from W1[:, :, chunk] + root1[:, chunk] (host-premixed lhsT: (A_r h)^T per
  relation + h^T for the root, so it is one 145-matmul PSUM accumulation).
- Layer 2: hidden-dim contraction sharding. Core c computes the partial
  P_c = sum_r (A_r h1[:, chunk]) @ W2_r[chunk, :] + h1[:, chunk] @ root2[chunk, :]
  over the h1 columns it already owns — no inter-core exchange. The host
  sums the 8 [17, 2048] partials and adds bias2.
Per-core HBM traffic is the roofline term: 37.75 MB of fp32 weights,
streamed as contiguous 2 MB slabs (16 KB per partition per DMA).
"""
import sys

if '/opt/trn_rl_repo' not in sys.path:
    sys.path.insert(0, '/opt/trn_rl_repo')

import numpy as np
from concourse import bacc, tile, mybir, bass_utils

N_NODES = 17
N_REL = 8
D = 2048
N_CORES = 8
CH = D // N_CORES          # 256 columns of h1 owned per core
KT = 128                    # contraction rows per matmul
JT = D // KT                # 16 k-tiles per layer-1 slab
NSTRIP = 4                  # layer-2 output strips of 512 columns
F32 = mybir.dt.float32

# packed const-tensor layout (fp32 word offsets)
NX = 9 * JT * N_NODES       # 2448: layer-1 lhsT
OFF_AT = NX                 # [0:17, ...]: A_r^T stack (136)
OFF_ID = NX + N_REL * N_NODES          # [0:17, ...]: identity (17)
OFF_B1 = 2608               # [0:1, ...]: bias1 chunk (256)
OFF_ONES = 2880             # [0:1, ...]: ones (17)
CONST_W = 2912

_compiled = None


def _build():
    nc = bacc.Bacc("TRN2", target_bir_lowering=False, debug=False,
                   num_devices=N_CORES)
    # layer-1 weights: 9 slabs [128, 16*256], K-permuted (partition p holds
    # rows {16p+j}); layer-2 weights: 9 slabs [128, 2*2048] (partition p
    # holds rows p and 128+p of the core's 256-row band).
    w1 = nc.dram_tensor("w1", [9, KT, JT * CH], F32,
                        kind="ExternalInput").ap()
    w2 = nc.dram_tensor("w2", [9, KT, 2 * D], F32,
                        kind="ExternalInput").ap()
    # all small operands packed into one tensor / one DMA (tiny transfers
    # starve behind the 2 MB slab DMAs otherwise)
    consts = nc.dram_tensor("consts", [KT, CONST_W], F32,
                            kind="ExternalInput").ap()
    out = nc.dram_tensor("out", [N_NODES, D], F32,
                         kind="ExternalOutput").ap()

    with tile.TileContext(nc) as tc:
        with tc.tile_pool(name="const", bufs=1) as constp, \
             tc.tile_pool(name="wpool", bufs=8) as wpool, \
             tc.tile_pool(name="spool", bufs=2) as spool, \
             tc.tile_pool(name="opsum", bufs=1, space="PSUM") as opsum, \
             tc.tile_pool(name="ppsum", bufs=2, space="PSUM") as ppsum:

            const_sb = constp.tile([KT, CONST_W], F32)
            nc.scalar.dma_start(out=const_sb, in_=consts)
            x1t_sb = const_sb[:, 0:NX]
            at_sb = const_sb[0:N_NODES, OFF_AT:OFF_AT + N_REL * N_NODES]
            id_sb = const_sb[0:N_NODES, OFF_ID:OFF_ID + N_NODES]
            b1_sb = const_sb[0:1, OFF_B1:OFF_B1 + CH]
            ones_sb = const_sb[0:1, OFF_ONES:OFF_ONES + N_NODES]

            # ---------------- layer 1 ----------------
            out1 = opsum.tile([N_NODES, CH], F32, name="out1")
            nc.tensor.matmul(out1, lhsT=ones_sb, rhs=b1_sb,
                             start=True, stop=False)
            for s in range(9):
                w = wpool.tile([KT, JT * CH], F32, name="wslab", tag="wslab")
                if s == 0:
                    # quarter the first slab so PE can start ~4 us earlier
                    for q in range(4):
                        nc.sync.dma_start(
                            out=w[:, q * 4 * CH:(q + 1) * 4 * CH],
                            in_=w1[s][:, q * 4 * CH:(q + 1) * 4 * CH])
                else:
                    nc.sync.dma_start(out=w, in_=w1[s])
                for j in range(JT):
                    k = s * JT + j
                    nc.tensor.matmul(
                        out1,
                        lhsT=x1t_sb[:, k * N_NODES:(k + 1) * N_NODES],
                        rhs=w[:, j * CH:(j + 1) * CH],
                        start=False, stop=(s == 8 and j == JT - 1))
            h1 = spool.tile([N_NODES, CH], F32, name="h1")
            nc.scalar.activation(h1, out1, mybir.ActivationFunctionType.Relu)

            # layer-2 lhsT prep: (A_r h1_c)^T tiles for r<8, h1_c^T for root
            xt2_sb = spool.tile([KT, 18 * N_NODES], F32, name="xt2_sb")
            for s in range(9):
                rhs = (at_sb[:, s * N_NODES:(s + 1) * N_NODES]
                       if s < N_REL else id_sb)
                for kt in range(2):
                    pp = ppsum.tile([KT, N_NODES], F32, name="pp")
                    nc.tensor.matmul(pp, lhsT=h1[:, kt * KT:(kt + 1) * KT],
                                     rhs=rhs, start=True, stop=True)
                    i = s * 2 + kt
                    nc.vector.tensor_copy(
                        xt2_sb[:, i * N_NODES:(i + 1) * N_NODES], pp)

            # ---------------- layer 2 (partial over owned h1 columns) -----
            out2 = [None] * NSTRIP
            for n in range(NSTRIP):
                t = opsum.tile([N_NODES, 512], F32, name=f"out2_{n}",
                               tag=f"out2_{n}")
                out2[n] = t
            for s in range(9):
                w = wpool.tile([KT, 2 * D], F32, name="wslab", tag="wslab")
                nc.sync.dma_start(out=w, in_=w2[s])
                for kt in range(2):
                    i = s * 2 + kt
                    lhsT = xt2_sb[:, i * N_NODES:(i + 1) * N_NODES]
                    for n in range(NSTRIP):
                        nc.tensor.matmul(
                            out2[n], lhsT=lhsT,
                            rhs=w[:, kt * D + n * 512: kt * D + (n + 1) * 512],
                            start=(s == 0 and kt == 0),
                            stop=(s == 8 and kt == 1),
                            skip_group_check=True)
            osb = spool.tile([N_NODES, D], F32, name="osb")
            for n in range(NSTRIP):
                nc.vector.tensor_copy(osb[:, n * 512:(n + 1) * 512], out2[n])
            nc.scalar.dma_start(out=out, in_=osb)

    nc.compile()
    return nc


def _prep_inputs(inputs):
    """Host-side prep: A matrices, premixed layer-1 lhsT, per-core weights."""
    h = np.array(inputs['node_emb'], dtype=np.float32, copy=True)
    sf = np.asarray(inputs['signal_features'], dtype=np.float32)
    h[:sf.shape[0]] = sf
    src = np.asarray(inputs['edge_index'])[0].astype(np.int64)
    dst = np.asarray(inputs['edge_index'])[1].astype(np.int64)
    et = np.asarray(inputs['edge_type']).astype(np.int64)

    A = np.zeros((N_REL, N_NODES, N_NODES), np.float32)
    cnt = np.zeros((N_REL, N_NODES), np.float32)
    np.add.at(cnt, (et, dst), 1.0)
    np.add.at(A, (et, dst, src), 1.0)
    A /= np.maximum(cnt, 1.0)[:, :, None]

    # layer-1 lhsT: 9 slabs of (A_r h)^T (+ h^T for root), K-permuted so
    # partition p holds rows {16p+j}: [128, 9*16*17]
    Z = np.concatenate([np.einsum('rij,jd->rid', A, h).astype(np.float32),
                        h[None]], axis=0)           # [9,17,2048]
    x1t = (Z.transpose(0, 2, 1)                      # [9,2048,17]
            .reshape(9, KT, JT, N_NODES)
            .transpose(1, 0, 2, 3)
            .reshape(KT, 9 * JT * N_NODES)).astype(np.float32).copy()

    # A_r^T stacked along columns: at[n, r*17+m] = A[r][m, n]
    at = (A.transpose(0, 2, 1).transpose(1, 0, 2)
           .reshape(N_NODES, N_REL * N_NODES)).astype(np.float32).copy()

    W1 = np.asarray(inputs['W1'], dtype=np.float32)
    W2 = np.asarray(inputs['W2'], dtype=np.float32)
    r1 = np.asarray(inputs['root1'], dtype=np.float32)
    r2 = np.asarray(inputs['root2'], dtype=np.float32)
    bias1 = np.asarray(inputs['bias1'], dtype=np.float32)
    W1full = np.concatenate([W1, r1[None]], axis=0)   # [9,2048,2048]
    W2full = np.concatenate([W2, r2[None]], axis=0)   # [9,2048,2048]

    consts = np.zeros((KT, CONST_W), np.float32)
    consts[:, 0:NX] = x1t
    consts[0:N_NODES, OFF_AT:OFF_AT + N_REL * N_NODES] = at
    consts[0:N_NODES, OFF_ID:OFF_ID + N_NODES] = np.eye(N_NODES)
    consts[0, OFF_ONES:OFF_ONES + N_NODES] = 1.0

    in_maps = []
    for c in range(N_CORES):
        cols = slice(c * CH, (c + 1) * CH)
        w1c = (W1full[:, :, cols]
               .reshape(9, KT, JT, CH)
               .reshape(9, KT, JT * CH)).astype(np.float32).copy()
        w2c = (W2full[:, cols, :]
               .reshape(9, 2, KT, D)
               .transpose(0, 2, 1, 3)
               .reshape(9, KT, 2 * D)).astype(np.float32).copy()
        cc = consts.copy()
        cc[0, OFF_B1:OFF_B1 + CH] = bias1[cols]
        in_maps.append({
            'w1': w1c,
            'w2': w2c,
            'consts': cc,
        })
    return in_maps


def get_compiled():
    global _compiled
    if _compiled is None:
        _compiled = _build()
    return _compiled


def run(inputs, trace=False):
    nc = get_compiled()
    in_maps = _prep_inputs(inputs)
    res = bass_utils.run_bass_kernel_spmd(
        nc, in_maps, core_ids=list(range(N_CORES)), trace=trace)
    acc = np.zeros((N_NODES, D), np.float64)
    for c in range(N_CORES):
        acc += np.asarray(res.results[c]['out'], dtype=np.float64)
    acc += np.asarray(inputs['bias2'], dtype=np.float64)[None, :]
    return acc.astype(np.float32), res


def kernel(**inputs):
    outp, _ = run(inputs, trace=False)
    return outp


# revision 8
# speedup vs baseline: 1.6051x; 1.0947x over previous
"""Trainium2 Bass kernel for BrainInspiredEmotionGraph (2-layer RGCN, 17 nodes,
8 relations, d=2048) running SPMD on 8 NeuronCores.

Math: layer(x) = sum_r A_r @ x @ W_r + x @ root + bias, where A_r is the
[17,17] per-relation mean-aggregation matrix built from the edge list.
h1 = relu(layer1(h)); out = layer2(h1), h = node_emb with signal rows patched.

Sharding (fully collective-free):
- Layer 1: output-column sharding. Core c computes h1[:, c*256:(c+1)*256]
  from W1[:, :, chunk] + root1[:, chunk] (host-premixed lhsT: (A_r h)^T per
  relation + h^T for the root, one long PSUM accumulation).
- Layer 2: hidden-dim contraction sharding. Core c computes the partial
  P_c = sum_r (A_r h1[:, chunk]) @ W2_r[chunk, :] + h1[:, chunk] @ root2[chunk, :]
  over the h1 columns it already owns — no inter-core exchange. The host
  sums the 8 [17, 2048] partials and adds bias2.

Precision/speed: every fp32 weight (and the layer-1 lhsT) is split on the
host into a bf16 (hi, lo) pair — identical HBM bytes, but each K-tile runs
as 3 bf16 matmuls (hi*hi + lo*hi + hi*lo, the lo*lo term is ~2^-16 and
dropped) at 1 cycle/row instead of fp32's 4, with fp32 PSUM accumulation.
Per-core HBM traffic is the roofline term: 37.75 MB streamed as contiguous
2 MB slabs (16 KB per partition per DMA).
"""
import sys

if '/opt/trn_rl_repo' not in sys.path:
    sys.path.insert(0, '/opt/trn_rl_repo')

import numpy as np
import ml_dtypes
from concourse import bacc, tile, mybir, bass_utils

BF16 = ml_dtypes.bfloat16
N_NODES = 17
N_REL = 8
D = 2048
N_CORES = 8
CH = D // N_CORES          # 256 columns of h1 owned per core
KT = 128                    # contraction rows per matmul
JT = D // KT                # 16 k-tiles per layer-1 slab
NSTRIP = 4                  # layer-2 output strips of 512 columns
F32 = mybir.dt.float32
BF = mybir.dt.bfloat16

NX = 9 * JT * N_NODES       # 2448 lhsT columns per hi/lo half
# fp32 const-tensor layout (word offsets): A_r^T stack, identity, b1, ones
OFF_AT = 0
OFF_ID = N_REL * N_NODES
OFF_B1 = 160
OFF_ONES = 416
CONSTF_W = 448

_compiled = None


def _build():
    nc = bacc.Bacc("TRN2", target_bir_lowering=False, debug=False,
                   num_devices=N_CORES)
    # layer-1 slabs: [128, 16 j-tiles * (hi,lo) * 256] bf16, K-permuted
    # (partition p holds rows {16p+j}); layer-2 slabs: [128, 2 kt * (hi,lo)
    # * 2048] bf16 (partition p holds rows p and 128+p of the 256-row band).
    w1 = nc.dram_tensor("w1", [9, KT, JT * 2 * CH], BF,
                        kind="ExternalInput").ap()
    w2 = nc.dram_tensor("w2", [9, KT, 4 * D], BF,
                        kind="ExternalInput").ap()
    xhl = nc.dram_tensor("xhl", [KT, 2 * NX], BF,
                         kind="ExternalInput").ap()
    cf = nc.dram_tensor("cf", [N_NODES, CONSTF_W], F32,
                        kind="ExternalInput").ap()
    out = nc.dram_tensor("out", [N_NODES, D], F32,
                         kind="ExternalOutput").ap()

    with tile.TileContext(nc) as tc:
        with tc.tile_pool(name="const", bufs=1) as constp, \
             tc.tile_pool(name="wpool", bufs=8) as wpool, \
             tc.tile_pool(name="spool", bufs=2) as spool, \
             tc.tile_pool(name="opsum", bufs=1, space="PSUM") as opsum, \
             tc.tile_pool(name="ppsum", bufs=2, space="PSUM") as ppsum:

            xhl_sb = constp.tile([KT, 2 * NX], BF)
            nc.scalar.dma_start(out=xhl_sb, in_=xhl)
            cf_sb = constp.tile([N_NODES, CONSTF_W], F32)
            nc.scalar.dma_start(out=cf_sb, in_=cf)
            at_sb = cf_sb[:, OFF_AT:OFF_AT + N_REL * N_NODES]
            id_sb = cf_sb[:, OFF_ID:OFF_ID + N_NODES]
            b1_sb = cf_sb[0:1, OFF_B1:OFF_B1 + CH]
            ones_sb = cf_sb[0:1, OFF_ONES:OFF_ONES + N_NODES]

            def xh(k):
                return xhl_sb[:, k * N_NODES:(k + 1) * N_NODES]

            def xl(k):
                return xhl_sb[:, NX + k * N_NODES:NX + (k + 1) * N_NODES]

            # ---------------- layer 1 ----------------
            out1 = opsum.tile([N_NODES, CH], F32, name="out1")
            nc.tensor.matmul(out1, lhsT=ones_sb, rhs=b1_sb,
                             start=True, stop=False)
            for s in range(9):
                w = wpool.tile([KT, JT * 2 * CH], BF, name="wslab",
                               tag="wslab")
                if s == 0:
                    # quarter the first slab so PE can start earlier
                    q4 = JT * 2 * CH // 4
                    for q in range(4):
                        nc.sync.dma_start(out=w[:, q * q4:(q + 1) * q4],
                                          in_=w1[s][:, q * q4:(q + 1) * q4])
                else:
                    nc.sync.dma_start(out=w, in_=w1[s])
                for j in range(JT):
                    k = s * JT + j
                    whi = w[:, (2 * j) * CH:(2 * j + 1) * CH]
                    wlo = w[:, (2 * j + 1) * CH:(2 * j + 2) * CH]
                    last = (s == 8 and j == JT - 1)
                    nc.tensor.matmul(out1, lhsT=xh(k), rhs=whi,
                                     start=False, stop=False)
                    nc.tensor.matmul(out1, lhsT=xl(k), rhs=whi,
                                     start=False, stop=False)
                    nc.tensor.matmul(out1, lhsT=xh(k), rhs=wlo,
                                     start=False, stop=last)
            h1 = spool.tile([N_NODES, CH], F32, name="h1")
            nc.scalar.activation(h1, out1, mybir.ActivationFunctionType.Relu)

            # layer-2 lhsT prep: (A_r h1_c)^T for r<8 + h1_c^T for the root,
            # each split into bf16 hi/lo tiles
            xt2_hi = spool.tile([KT, 18 * N_NODES], BF, name="xt2_hi")
            xt2_lo = spool.tile([KT, 18 * N_NODES], BF, name="xt2_lo")
            for s in range(9):
                rhs = (at_sb[:, s * N_NODES:(s + 1) * N_NODES]
                       if s < N_REL else id_sb)
                for kt in range(2):
                    i = s * 2 + kt
                    sl = slice(i * N_NODES, (i + 1) * N_NODES)
                    pp = ppsum.tile([KT, N_NODES], F32, name="pp")
                    nc.tensor.matmul(pp, lhsT=h1[:, kt * KT:(kt + 1) * KT],
                                     rhs=rhs, start=True, stop=True)
                    nc.vector.tensor_copy(xt2_hi[:, sl], pp)
                    hi32 = spool.tile([KT, N_NODES], F32, name="hi32")
                    nc.vector.tensor_copy(hi32, xt2_hi[:, sl])
                    nc.vector.tensor_sub(xt2_lo[:, sl], pp, hi32)

            # ---------------- layer 2 (partial over owned h1 columns) -----
            out2 = []
            for n in range(NSTRIP):
                out2.append(opsum.tile([N_NODES, 512], F32, name=f"out2_{n}",
                                       tag=f"out2_{n}"))
            for s in range(9):
                w = wpool.tile([KT, 4 * D], BF, name="wslab", tag="wslab")
                nc.sync.dma_start(out=w, in_=w2[s])
                for kt in range(2):
                    i = s * 2 + kt
                    lhi = xt2_hi[:, i * N_NODES:(i + 1) * N_NODES]
                    llo = xt2_lo[:, i * N_NODES:(i + 1) * N_NODES]
                    for n in range(NSTRIP):
                        whi = w[:, (2 * kt) * D + n * 512:
                                (2 * kt) * D + (n + 1) * 512]
                        wlo = w[:, (2 * kt + 1) * D + n * 512:
                                (2 * kt + 1) * D + (n + 1) * 512]
                        first = (s == 0 and kt == 0)
                        last = (s == 8 and kt == 1)
                        nc.tensor.matmul(out2[n], lhsT=lhi, rhs=whi,
                                         start=first, stop=False,
                                         skip_group_check=True)
                        nc.tensor.matmul(out2[n], lhsT=llo, rhs=whi,
                                         start=False, stop=False,
                                         skip_group_check=True)
                        nc.tensor.matmul(out2[n], lhsT=lhi, rhs=wlo,
                                         start=False, stop=last,
                                         skip_group_check=True)
            osb = spool.tile([N_NODES, D], F32, name="osb")
            for n in range(NSTRIP):
                nc.vector.tensor_copy(osb[:, n * 512:(n + 1) * 512], out2[n])
            nc.scalar.dma_start(out=out, in_=osb)

    nc.compile()
    return nc


def _hilo(w):
    """Split fp32 array into bf16 (hi, lo)."""
    hi = w.astype(BF16)
    lo = (w - hi.astype(np.float32)).astype(BF16)
    return hi, lo


def _prep_inputs(inputs):
    """Host-side prep: A matrices, premixed layer-1 lhsT, per-core weights."""
    h = np.array(inputs['node_emb'], dtype=np.float32, copy=True)
    sf = np.asarray(inputs['signal_features'], dtype=np.float32)
    h[:sf.shape[0]] = sf
    src = np.asarray(inputs['edge_index'])[0].astype(np.int64)
    dst = np.asarray(inputs['edge_index'])[1].astype(np.int64)
    et = np.asarray(inputs['edge_type']).astype(np.int64)

    A = np.zeros((N_REL, N_NODES, N_NODES), np.float32)
    cnt = np.zeros((N_REL, N_NODES), np.float32)
    np.add.at(cnt, (et, dst), 1.0)
    np.add.at(A, (et, dst, src), 1.0)
    A /= np.maximum(cnt, 1.0)[:, :, None]

    # layer-1 lhsT: 9 slabs of (A_r h)^T (+ h^T for root), K-permuted so
    # partition p holds rows {16p+j}: [128, 2448] fp32 -> bf16 hi/lo halves
    Z = np.concatenate([np.einsum('rij,jd->rid', A, h).astype(np.float32),
                        h[None]], axis=0)           # [9,17,2048]
    x1t = (Z.transpose(0, 2, 1)
            .reshape(9, KT, JT, N_NODES)
            .transpose(1, 0, 2, 3)
            .reshape(KT, NX)).astype(np.float32)
    xhi, xlo = _hilo(x1t)
    xhl = np.concatenate([xhi, xlo], axis=1).copy()  # [128, 2*NX] bf16

    # A_r^T stacked along columns: at[n, r*17+m] = A[r][m, n]
    at = (A.transpose(0, 2, 1).transpose(1, 0, 2)
           .reshape(N_NODES, N_REL * N_NODES)).astype(np.float32)

    W1 = np.asarray(inputs['W1'], dtype=np.float32)
    W2 = np.asarray(inputs['W2'], dtype=np.float32)
    r1 = np.asarray(inputs['root1'], dtype=np.float32)
    r2 = np.asarray(inputs['root2'], dtype=np.float32)
    bias1 = np.asarray(inputs['bias1'], dtype=np.float32)
    W1full = np.concatenate([W1, r1[None]], axis=0)   # [9,2048,2048]
    W2full = np.concatenate([W2, r2[None]], axis=0)   # [9,2048,2048]

    cf = np.zeros((N_NODES, CONSTF_W), np.float32)
    cf[:, OFF_AT:OFF_AT + N_REL * N_NODES] = at
    cf[:, OFF_ID:OFF_ID + N_NODES] = np.eye(N_NODES)
    cf[0, OFF_ONES:OFF_ONES + N_NODES] = 1.0

    in_maps = []
    for c in range(N_CORES):
        cols = slice(c * CH, (c + 1) * CH)
        w1c = (W1full[:, :, cols]
               .reshape(9, KT, JT, CH))               # [9,128,16,256] f32
        h1c, l1c = _hilo(w1c)
        w1hl = (np.stack([h1c, l1c], axis=3)          # [9,128,16,2,256]
                .reshape(9, KT, JT * 2 * CH)).copy()
        w2c = (W2full[:, cols, :]
               .reshape(9, 2, KT, D)
               .transpose(0, 2, 1, 3))                # [9,128,2,2048] f32
        h2c, l2c = _hilo(w2c)
        w2hl = (np.stack([h2c, l2c], axis=3)          # [9,128,2,2,2048]
                .reshape(9, KT, 4 * D)).copy()
        cfc = cf.copy()
        cfc[0, OFF_B1:OFF_B1 + CH] = bias1[cols]
        in_maps.append({
            'w1': w1hl,
            'w2': w2hl,
            'xhl': xhl,
            'cf': cfc,
        })
    return in_maps


def get_compiled():
    global _compiled
    if _compiled is None:
        _compiled = _build()
    return _compiled


def run(inputs, trace=False):
    nc = get_compiled()
    in_maps = _prep_inputs(inputs)
    res = bass_utils.run_bass_kernel_spmd(
        nc, in_maps, core_ids=list(range(N_CORES)), trace=trace)
    acc = np.zeros((N_NODES, D), np.float64)
    for c in range(N_CORES):
        acc += np.asarray(res.results[c]['out'], dtype=np.float64)
    acc += np.asarray(inputs['bias2'], dtype=np.float64)[None, :]
    return acc.astype(np.float32), res


def kernel(**inputs):
    outp, _ = run(inputs, trace=False)
    return outp


# revision 11
# speedup vs baseline: 1.7143x; 1.0680x over previous
"""Trainium2 Bass kernel for BrainInspiredEmotionGraph (2-layer RGCN, 17 nodes,
8 relations, d=2048) running SPMD on 8 NeuronCores.

Math: layer(x) = sum_r A_r @ x @ W_r + x @ root + bias, where A_r is the
[17,17] per-relation mean-aggregation matrix built from the edge list.
h1 = relu(layer1(h)); out = layer2(h1), h = node_emb with signal rows patched.

Sharding (fully collective-free):
- Layer 1: output-column sharding. Core c computes h1[:, c*256:(c+1)*256]
  from W1[:, :, chunk] + root1[:, chunk] (host-premixed lhsT: (A_r h)^T per
  relation + h^T for the root, one long PSUM accumulation).
- Layer 2: hidden-dim contraction sharding. Core c computes the partial
  P_c = sum_r (A_r h1[:, chunk]) @ W2_r[chunk, :] + h1[:, chunk] @ root2[chunk, :]
  over the h1 columns it already owns — no inter-core exchange. The host
  sums the 8 [17, 2048] partials and adds bias2.

Precision/speed: every fp32 weight (and the layer-1 lhsT) is split on the
host into a bf16 (hi, lo) pair — identical HBM bytes, but each K-tile runs
as 3 bf16 matmuls (hi*hi + lo*hi + hi*lo, the lo*lo term is ~2^-16 and
dropped) at 1 cycle/row instead of fp32's 4, with fp32 PSUM accumulation.
Per-core HBM traffic is the roofline term: 37.75 MB streamed as contiguous
2 MB slabs (16 KB per partition per DMA).
"""
import sys

if '/opt/trn_rl_repo' not in sys.path:
    sys.path.insert(0, '/opt/trn_rl_repo')

import numpy as np
import ml_dtypes
from concourse import bacc, tile, mybir, bass_utils

BF16 = ml_dtypes.bfloat16
N_NODES = 17
N_REL = 8
D = 2048
N_CORES = 8
CH = D // N_CORES          # 256 columns of h1 owned per core
KT = 128                    # contraction rows per matmul
JT = D // KT                # 16 k-tiles per layer-1 slab
NSTRIP = 4                  # layer-2 output strips of 512 columns
F32 = mybir.dt.float32
BF = mybir.dt.bfloat16

NX = 9 * JT * N_NODES       # 2448 lhsT columns per hi/lo half
# fp32 const-tensor layout (word offsets): A_r^T stack, identity, b1, ones
OFF_AT = 0
OFF_ID = N_REL * N_NODES
OFF_B1 = 160
OFF_ONES = 416
CONSTF_W = 448

_compiled = None


def _build():
    nc = bacc.Bacc("TRN2", target_bir_lowering=False, debug=False,
                   num_devices=N_CORES)
    # layer-1 slabs: [128, 16 j-tiles * (hi,lo) * 256] bf16, K-permuted
    # (partition p holds rows {16p+j}); layer-2 slabs: [128, 2 kt * (hi,lo)
    # * 2048] bf16 (partition p holds rows p and 128+p of the 256-row band).
    w1 = nc.dram_tensor("w1", [9, KT, JT * 2 * CH], BF,
                        kind="ExternalInput").ap()
    w2 = nc.dram_tensor("w2", [9, KT, 4 * D], BF,
                        kind="ExternalInput").ap()
    xhl = nc.dram_tensor("xhl", [KT, 2 * NX], BF,
                         kind="ExternalInput").ap()
    cf = nc.dram_tensor("cf", [N_NODES, CONSTF_W], F32,
                        kind="ExternalInput").ap()
    out = nc.dram_tensor("out", [N_NODES, D], F32,
                         kind="ExternalOutput").ap()

    with tile.TileContext(nc) as tc:
        with tc.tile_pool(name="const", bufs=1) as constp, \
             tc.tile_pool(name="wpool", bufs=8) as wpool, \
             tc.tile_pool(name="spool", bufs=2) as spool, \
             tc.tile_pool(name="opsum", bufs=1, space="PSUM") as opsum, \
             tc.tile_pool(name="ppsum", bufs=2, space="PSUM") as ppsum:

            cf_sb = constp.tile([N_NODES, CONSTF_W], F32)
            nc.scalar.dma_start(out=cf_sb, in_=cf)
            xhl_sb = constp.tile([KT, 2 * NX], BF)
            # split so the layer-1 slab-0 lhsT tiles land first
            nc.scalar.dma_start(out=xhl_sb[:, 0:JT * N_NODES],
                                in_=xhl[:, 0:JT * N_NODES])
            nc.scalar.dma_start(out=xhl_sb[:, NX:NX + JT * N_NODES],
                                in_=xhl[:, NX:NX + JT * N_NODES])
            nc.scalar.dma_start(out=xhl_sb[:, JT * N_NODES:NX],
                                in_=xhl[:, JT * N_NODES:NX])
            nc.scalar.dma_start(out=xhl_sb[:, NX + JT * N_NODES:],
                                in_=xhl[:, NX + JT * N_NODES:])
            at_sb = cf_sb[:, OFF_AT:OFF_AT + N_REL * N_NODES]
            id_sb = cf_sb[:, OFF_ID:OFF_ID + N_NODES]
            b1_sb = cf_sb[0:1, OFF_B1:OFF_B1 + CH]
            ones_sb = cf_sb[0:1, OFF_ONES:OFF_ONES + N_NODES]

            def xh(k):
                return xhl_sb[:, k * N_NODES:(k + 1) * N_NODES]

            def xl(k):
                return xhl_sb[:, NX + k * N_NODES:NX + (k + 1) * N_NODES]

            # ---------------- layer 1 ----------------
            out1 = opsum.tile([N_NODES, CH], F32, name="out1")
            nc.tensor.matmul(out1, lhsT=ones_sb, rhs=b1_sb,
                             start=True, stop=False)
            for s in range(9):
                w = wpool.tile([KT, JT * 2 * CH], BF, name="wslab",
                               tag="wslab")
                if s == 0:
                    # stream the first slab in eighths so PE starts earlier
                    q8 = JT * 2 * CH // 8
                    for q in range(8):
                        nc.sync.dma_start(out=w[:, q * q8:(q + 1) * q8],
                                          in_=w1[s][:, q * q8:(q + 1) * q8])
                else:
                    nc.sync.dma_start(out=w, in_=w1[s])
                for j in range(JT):
                    k = s * JT + j
                    whi = w[:, (2 * j) * CH:(2 * j + 1) * CH]
                    wlo = w[:, (2 * j + 1) * CH:(2 * j + 2) * CH]
                    last = (s == 8 and j == JT - 1)
                    nc.tensor.matmul(out1, lhsT=xh(k), rhs=whi,
                                     start=False, stop=False)
                    nc.tensor.matmul(out1, lhsT=xl(k), rhs=whi,
                                     start=False, stop=False)
                    nc.tensor.matmul(out1, lhsT=xh(k), rhs=wlo,
                                     start=False, stop=last)
            h1 = spool.tile([N_NODES, CH], F32, name="h1")
            nc.scalar.activation(h1, out1, mybir.ActivationFunctionType.Relu)

            # layer-2 lhsT prep: (A_r h1_c)^T for r<8 + h1_c^T for the root,
            # each split into bf16 hi/lo tiles
            xt2_hi = spool.tile([KT, 18 * N_NODES], BF, name="xt2_hi")
            xt2_lo = spool.tile([KT, 18 * N_NODES], BF, name="xt2_lo")
            for s in range(9):
                rhs = (at_sb[:, s * N_NODES:(s + 1) * N_NODES]
                       if s < N_REL else id_sb)
                for kt in range(2):
                    i = s * 2 + kt
                    sl = slice(i * N_NODES, (i + 1) * N_NODES)
                    pp = ppsum.tile([KT, N_NODES], F32, name="pp")
                    nc.tensor.matmul(pp, lhsT=h1[:, kt * KT:(kt + 1) * KT],
                                     rhs=rhs, start=True, stop=True)
                    nc.vector.tensor_copy(xt2_hi[:, sl], pp)
                    hi32 = spool.tile([KT, N_NODES], F32, name="hi32")
                    nc.vector.tensor_copy(hi32, xt2_hi[:, sl])
                    nc.vector.tensor_sub(xt2_lo[:, sl], pp, hi32)

            # ---------------- layer 2 (partial over owned h1 columns) -----
            out2 = []
            for n in range(NSTRIP):
                out2.append(opsum.tile([N_NODES, 512], F32, name=f"out2_{n}",
                                       tag=f"out2_{n}"))
            for s in range(9):
                w = wpool.tile([KT, 4 * D], BF, name="wslab", tag="wslab")
                if s == 8:
                    # quarter the last slab so the tail pipelines
                    q4 = 4 * D // 4
                    for q in range(4):
                        nc.sync.dma_start(out=w[:, q * q4:(q + 1) * q4],
                                          in_=w2[s][:, q * q4:(q + 1) * q4])
                else:
                    nc.sync.dma_start(out=w, in_=w2[s])
                for kt in range(2):
                    i = s * 2 + kt
                    lhi = xt2_hi[:, i * N_NODES:(i + 1) * N_NODES]
                    llo = xt2_lo[:, i * N_NODES:(i + 1) * N_NODES]
                    for n in range(NSTRIP):
                        whi = w[:, (2 * kt) * D + n * 512:
                                (2 * kt) * D + (n + 1) * 512]
                        wlo = w[:, (2 * kt + 1) * D + n * 512:
                                (2 * kt + 1) * D + (n + 1) * 512]
                        first = (s == 0 and kt == 0)
                        last = (s == 8 and kt == 1)
                        nc.tensor.matmul(out2[n], lhsT=lhi, rhs=whi,
                                         start=first, stop=False,
                                         skip_group_check=True)
                        nc.tensor.matmul(out2[n], lhsT=llo, rhs=whi,
                                         start=False, stop=False,
                                         skip_group_check=True)
                        nc.tensor.matmul(out2[n], lhsT=lhi, rhs=wlo,
                                         start=False, stop=last,
                                         skip_group_check=True)
            osb = spool.tile([N_NODES, D], F32, name="osb")
            for n in range(NSTRIP):
                nc.vector.tensor_copy(osb[:, n * 512:(n + 1) * 512], out2[n])
            nc.scalar.dma_start(out=out, in_=osb)

    nc.compile()
    return nc


def _hilo(w):
    """Split fp32 array into bf16 (hi, lo)."""
    hi = w.astype(BF16)
    lo = (w - hi.astype(np.float32)).astype(BF16)
    return hi, lo


def _prep_inputs(inputs):
    """Host-side prep: A matrices, premixed layer-1 lhsT, per-core weights."""
    h = np.array(inputs['node_emb'], dtype=np.float32, copy=True)
    sf = np.asarray(inputs['signal_features'], dtype=np.float32)
    h[:sf.shape[0]] = sf
    src = np.asarray(inputs['edge_index'])[0].astype(np.int64)
    dst = np.asarray(inputs['edge_index'])[1].astype(np.int64)
    et = np.asarray(inputs['edge_type']).astype(np.int64)

    A = np.zeros((N_REL, N_NODES, N_NODES), np.float32)
    cnt = np.zeros((N_REL, N_NODES), np.float32)
    np.add.at(cnt, (et, dst), 1.0)
    np.add.at(A, (et, dst, src), 1.0)
    A /= np.maximum(cnt, 1.0)[:, :, None]

    # layer-1 lhsT: 9 slabs of (A_r h)^T (+ h^T for root), K-permuted so
    # partition p holds rows {16p+j}: [128, 2448] fp32 -> bf16 hi/lo halves
    Z = np.concatenate([np.einsum('rij,jd->rid', A, h).astype(np.float32),
                        h[None]], axis=0)           # [9,17,2048]
    x1t = (Z.transpose(0, 2, 1)
            .reshape(9, KT, JT, N_NODES)
            .transpose(1, 0, 2, 3)
            .reshape(KT, NX)).astype(np.float32)
    xhi, xlo = _hilo(x1t)
    xhl = np.concatenate([xhi, xlo], axis=1).copy()  # [128, 2*NX] bf16

    # A_r^T stacked along columns: at[n, r*17+m] = A[r][m, n]
    at = (A.transpose(0, 2, 1).transpose(1, 0, 2)
           .reshape(N_NODES, N_REL * N_NODES)).astype(np.float32)

    W1 = np.asarray(inputs['W1'], dtype=np.float32)
    W2 = np.asarray(inputs['W2'], dtype=np.float32)
    r1 = np.asarray(inputs['root1'], dtype=np.float32)
    r2 = np.asarray(inputs['root2'], dtype=np.float32)
    bias1 = np.asarray(inputs['bias1'], dtype=np.float32)
    W1full = np.concatenate([W1, r1[None]], axis=0)   # [9,2048,2048]
    W2full = np.concatenate([W2, r2[None]], axis=0)   # [9,2048,2048]

    cf = np.zeros((N_NODES, CONSTF_W), np.float32)
    cf[:, OFF_AT:OFF_AT + N_REL * N_NODES] = at
    cf[:, OFF_ID:OFF_ID + N_NODES] = np.eye(N_NODES)
    cf[0, OFF_ONES:OFF_ONES + N_NODES] = 1.0

    in_maps = []
    for c in range(N_CORES):
        cols = slice(c * CH, (c + 1) * CH)
        w1c = (W1full[:, :, cols]
               .reshape(9, KT, JT, CH))               # [9,128,16,256] f32
        h1c, l1c = _hilo(w1c)
        w1hl = (np.stack([h1c, l1c], axis=3)          # [9,128,16,2,256]
                .reshape(9, KT, JT * 2 * CH)).copy()
        w2c = (W2full[:, cols, :]
               .reshape(9, 2, KT, D)
               .transpose(0, 2, 1, 3))                # [9,128,2,2048] f32
        h2c, l2c = _hilo(w2c)
        w2hl = (np.stack([h2c, l2c], axis=3)          # [9,128,2,2,2048]
                .reshape(9, KT, 4 * D)).copy()
        cfc = cf.copy()
        cfc[0, OFF_B1:OFF_B1 + CH] = bias1[cols]
        in_maps.append({
            'w1': w1hl,
            'w2': w2hl,
            'xhl': xhl,
            'cf': cfc,
        })
    return in_maps


def get_compiled():
    global _compiled
    if _compiled is None:
        _compiled = _build()
    return _compiled


def run(inputs, trace=False):
    nc = get_compiled()
    in_maps = _prep_inputs(inputs)
    res = bass_utils.run_bass_kernel_spmd(
        nc, in_maps, core_ids=list(range(N_CORES)), trace=trace)
    acc = np.zeros((N_NODES, D), np.float64)
    for c in range(N_CORES):
        acc += np.asarray(res.results[c]['out'], dtype=np.float64)
    acc += np.asarray(inputs['bias2'], dtype=np.float64)[None, :]
    return acc.astype(np.float32), res


def kernel(**inputs):
    outp, _ = run(inputs, trace=False)
    return outp


# revision 15
# speedup vs baseline: 1.7635x; 1.0287x over previous
"""Trainium2 Bass kernel for BrainInspiredEmotionGraph (2-layer RGCN, 17 nodes,
8 relations, d=2048) running SPMD on 8 NeuronCores.

Math: layer(x) = sum_r A_r @ x @ W_r + x @ root + bias, where A_r is the
[17,17] per-relation mean-aggregation matrix built from the edge list.
h1 = relu(layer1(h)); out = layer2(h1), h = node_emb with signal rows patched.

Sharding (fully collective-free):
- Layer 1: output-column sharding. Core c computes h1[:, c*256:(c+1)*256]
  from W1[:, :, chunk] + root1[:, chunk] (host-premixed lhsT: (A_r h)^T per
  relation + h^T for the root, one long PSUM accumulation).
- Layer 2: hidden-dim contraction sharding. Core c computes the partial
  P_c = sum_r (A_r h1[:, chunk]) @ W2_r[chunk, :] + h1[:, chunk] @ root2[chunk, :]
  over the h1 columns it already owns — no inter-core exchange. The host
  sums the 8 [17, 2048] partials and adds bias2.

Precision/speed: every fp32 weight (and the layer-1 lhsT) is split on the
host into a bf16 (hi, lo) pair — identical HBM bytes, but each K-tile runs
as 3 bf16 matmuls (hi*hi + lo*hi + hi*lo, the lo*lo term is ~2^-16 and
dropped) at 1 cycle/row instead of fp32's 4, with fp32 PSUM accumulation.
Per-core HBM traffic is the roofline term: 37.75 MB streamed as contiguous
2 MB slabs (16 KB per partition per DMA).
"""
import sys

if '/opt/trn_rl_repo' not in sys.path:
    sys.path.insert(0, '/opt/trn_rl_repo')

import numpy as np
import ml_dtypes
from concourse import bacc, tile, mybir, bass_utils

BF16 = ml_dtypes.bfloat16
N_NODES = 17
N_REL = 8
D = 2048
N_CORES = 8
CH = D // N_CORES          # 256 columns of h1 owned per core
KT = 128                    # contraction rows per matmul
JT = D // KT                # 16 k-tiles per layer-1 slab
NSTRIP = 4                  # layer-2 output strips of 512 columns
F32 = mybir.dt.float32
BF = mybir.dt.bfloat16

NX = 9 * JT * N_NODES       # 2448 lhsT columns per hi/lo half
# fp32 const-tensor layout (word offsets): A_r^T stack, identity, b1, ones
OFF_AT = 0
OFF_ID = N_REL * N_NODES
OFF_B1 = 160
OFF_ONES = 416
CONSTF_W = 448

_compiled = None


def _build():
    nc = bacc.Bacc("TRN2", target_bir_lowering=False, debug=False,
                   num_devices=N_CORES)
    # layer-1 slabs: [128, 16 j-tiles * (hi,lo) * 256] bf16, K-permuted
    # (partition p holds rows {16p+j}); layer-2 slabs: [128, 2 kt * (hi,lo)
    # * 2048] bf16 (partition p holds rows p and 128+p of the 256-row band).
    w1 = nc.dram_tensor("w1", [9, KT, JT * 2 * CH], BF,
                        kind="ExternalInput").ap()
    w2 = nc.dram_tensor("w2", [9, KT, 4 * D], BF,
                        kind="ExternalInput").ap()
    xhl = nc.dram_tensor("xhl", [KT, 2 * NX], BF,
                         kind="ExternalInput").ap()
    cf = nc.dram_tensor("cf", [N_NODES, CONSTF_W], F32,
                        kind="ExternalInput").ap()
    out = nc.dram_tensor("out", [KT, JT * N_NODES], F32,
                         kind="ExternalOutput").ap()

    with tile.TileContext(nc) as tc:
        with tc.tile_pool(name="const", bufs=1) as constp, \
             tc.tile_pool(name="wpool", bufs=8) as wpool, \
             tc.tile_pool(name="spool", bufs=2) as spool, \
             tc.tile_pool(name="opsum", bufs=1, space="PSUM") as opsum, \
             tc.tile_pool(name="ppsum", bufs=2, space="PSUM") as ppsum:

            cf_sb = constp.tile([N_NODES, CONSTF_W], F32)
            nc.scalar.dma_start(out=cf_sb, in_=cf)
            xhl_sb = constp.tile([KT, 2 * NX], BF)
            # split so the layer-1 slab-0 lhsT tiles land first
            nc.scalar.dma_start(out=xhl_sb[:, 0:JT * N_NODES],
                                in_=xhl[:, 0:JT * N_NODES])
            nc.scalar.dma_start(out=xhl_sb[:, NX:NX + JT * N_NODES],
                                in_=xhl[:, NX:NX + JT * N_NODES])
            nc.scalar.dma_start(out=xhl_sb[:, JT * N_NODES:NX],
                                in_=xhl[:, JT * N_NODES:NX])
            nc.scalar.dma_start(out=xhl_sb[:, NX + JT * N_NODES:],
                                in_=xhl[:, NX + JT * N_NODES:])
            at_sb = cf_sb[:, OFF_AT:OFF_AT + N_REL * N_NODES]
            id_sb = cf_sb[:, OFF_ID:OFF_ID + N_NODES]
            b1_sb = cf_sb[0:1, OFF_B1:OFF_B1 + CH]
            ones_sb = cf_sb[0:1, OFF_ONES:OFF_ONES + N_NODES]

            def xh(k):
                return xhl_sb[:, k * N_NODES:(k + 1) * N_NODES]

            def xl(k):
                return xhl_sb[:, NX + k * N_NODES:NX + (k + 1) * N_NODES]

            # ---------------- layer 1 ----------------
            out1 = opsum.tile([N_NODES, CH], F32, name="out1")
            nc.tensor.matmul(out1, lhsT=ones_sb, rhs=b1_sb,
                             start=True, stop=False)
            for s in range(9):
                w = wpool.tile([KT, JT * 2 * CH], BF, name="wslab",
                               tag="wslab")
                if s == 0:
                    # stream the first slab in eighths so PE starts earlier
                    q8 = JT * 2 * CH // 8
                    for q in range(8):
                        nc.sync.dma_start(out=w[:, q * q8:(q + 1) * q8],
                                          in_=w1[s][:, q * q8:(q + 1) * q8])
                else:
                    nc.sync.dma_start(out=w, in_=w1[s])
                for j in range(JT):
                    k = s * JT + j
                    whi = w[:, (2 * j) * CH:(2 * j + 1) * CH]
                    wlo = w[:, (2 * j + 1) * CH:(2 * j + 2) * CH]
                    last = (s == 8 and j == JT - 1)
                    nc.tensor.matmul(out1, lhsT=xh(k), rhs=whi,
                                     start=False, stop=False)
                    nc.tensor.matmul(out1, lhsT=xl(k), rhs=whi,
                                     start=False, stop=False)
                    nc.tensor.matmul(out1, lhsT=xh(k), rhs=wlo,
                                     start=False, stop=last)
            h1 = spool.tile([N_NODES, CH], F32, name="h1")
            nc.scalar.activation(h1, out1, mybir.ActivationFunctionType.Relu)

            # layer-2 lhsT prep: (A_r h1_c)^T for r<8 + h1_c^T for the root,
            # each split into bf16 hi/lo tiles
            xt2_hi = spool.tile([KT, 18 * N_NODES], BF, name="xt2_hi")
            xt2_lo = spool.tile([KT, 18 * N_NODES], BF, name="xt2_lo")
            for s in range(9):
                rhs = (at_sb[:, s * N_NODES:(s + 1) * N_NODES]
                       if s < N_REL else id_sb)
                for kt in range(2):
                    i = s * 2 + kt
                    sl = slice(i * N_NODES, (i + 1) * N_NODES)
                    pp = ppsum.tile([KT, N_NODES], F32, name="pp")
                    nc.tensor.matmul(pp, lhsT=h1[:, kt * KT:(kt + 1) * KT],
                                     rhs=rhs, start=True, stop=True)
                    nc.vector.tensor_copy(xt2_hi[:, sl], pp)
                    hi32 = spool.tile([KT, N_NODES], F32, name="hi32")
                    nc.vector.tensor_copy(hi32, xt2_hi[:, sl])
                    nc.vector.tensor_sub(xt2_lo[:, sl], pp, hi32)

            # ---------------- layer 2 (partial over owned h1 columns) -----
            out2 = []
            for n in range(NSTRIP):
                out2.append(opsum.tile([N_NODES, 512], F32, name=f"out2_{n}",
                                       tag=f"out2_{n}"))
            for s in range(9):
                w = wpool.tile([KT, 4 * D], BF, name="wslab", tag="wslab")
                if s == 8:
                    # eighth the last slab so the tail pipelines
                    q8 = 4 * D // 8
                    for q in range(8):
                        nc.sync.dma_start(out=w[:, q * q8:(q + 1) * q8],
                                          in_=w2[s][:, q * q8:(q + 1) * q8])
                else:
                    nc.sync.dma_start(out=w, in_=w2[s])
                for kt in range(2):
                    i = s * 2 + kt
                    lhi = xt2_hi[:, i * N_NODES:(i + 1) * N_NODES]
                    llo = xt2_lo[:, i * N_NODES:(i + 1) * N_NODES]
                    for n in range(NSTRIP):
                        whi = w[:, (2 * kt) * D + n * 512:
                                (2 * kt) * D + (n + 1) * 512]
                        wlo = w[:, (2 * kt + 1) * D + n * 512:
                                (2 * kt + 1) * D + (n + 1) * 512]
                        first = (s == 0 and kt == 0)
                        last = (s == 8 and kt == 1)
                        nc.tensor.matmul(out2[n], lhsT=lhi, rhs=whi,
                                         start=first, stop=False,
                                         skip_group_check=True)
                        nc.tensor.matmul(out2[n], lhsT=llo, rhs=whi,
                                         start=False, stop=False,
                                         skip_group_check=True)
                        nc.tensor.matmul(out2[n], lhsT=lhi, rhs=wlo,
                                         start=False, stop=last,
                                         skip_group_check=True)
            # transpose the [17, 2048] result to a [128, 272] layout so the
            # final store uses all partitions (a 17-partition DMA costs ~7us)
            osb = spool.tile([N_NODES, D], F32, name="osb")
            osbt = spool.tile([KT, JT * N_NODES], F32, name="osbt")
            for n in range(NSTRIP):
                nc.vector.tensor_copy(osb[:, n * 512:(n + 1) * 512], out2[n])
                for qq in range(4):
                    q = n * 4 + qq
                    tq = ppsum.tile([KT, N_NODES], F32, name="pp", tag="pp")
                    nc.tensor.transpose(
                        tq, osb[:, q * KT:(q + 1) * KT], id_sb)
                    nc.vector.tensor_copy(
                        osbt[:, q * N_NODES:(q + 1) * N_NODES], tq)
            nc.scalar.dma_start(out=out, in_=osbt)

    nc.compile()
    return nc


def _hilo(w):
    """Split fp32 array into bf16 (hi, lo)."""
    hi = w.astype(BF16)
    lo = (w - hi.astype(np.float32)).astype(BF16)
    return hi, lo


def _prep_inputs(inputs):
    """Host-side prep: A matrices, premixed layer-1 lhsT, per-core weights."""
    h = np.array(inputs['node_emb'], dtype=np.float32, copy=True)
    sf = np.asarray(inputs['signal_features'], dtype=np.float32)
    h[:sf.shape[0]] = sf
    src = np.asarray(inputs['edge_index'])[0].astype(np.int64)
    dst = np.asarray(inputs['edge_index'])[1].astype(np.int64)
    et = np.asarray(inputs['edge_type']).astype(np.int64)

    A = np.zeros((N_REL, N_NODES, N_NODES), np.float32)
    cnt = np.zeros((N_REL, N_NODES), np.float32)
    np.add.at(cnt, (et, dst), 1.0)
    np.add.at(A, (et, dst, src), 1.0)
    A /= np.maximum(cnt, 1.0)[:, :, None]

    # layer-1 lhsT: 9 slabs of (A_r h)^T (+ h^T for root), K-permuted so
    # partition p holds rows {16p+j}: [128, 2448] fp32 -> bf16 hi/lo halves
    Z = np.concatenate([np.einsum('rij,jd->rid', A, h).astype(np.float32),
                        h[None]], axis=0)           # [9,17,2048]
    x1t = (Z.transpose(0, 2, 1)
            .reshape(9, KT, JT, N_NODES)
            .transpose(1, 0, 2, 3)
            .reshape(KT, NX)).astype(np.float32)
    xhi, xlo = _hilo(x1t)
    xhl = np.concatenate([xhi, xlo], axis=1).copy()  # [128, 2*NX] bf16

    # A_r^T stacked along columns: at[n, r*17+m] = A[r][m, n]
    at = (A.transpose(0, 2, 1).transpose(1, 0, 2)
           .reshape(N_NODES, N_REL * N_NODES)).astype(np.float32)

    W1 = np.asarray(inputs['W1'], dtype=np.float32)
    W2 = np.asarray(inputs['W2'], dtype=np.float32)
    r1 = np.asarray(inputs['root1'], dtype=np.float32)
    r2 = np.asarray(inputs['root2'], dtype=np.float32)
    bias1 = np.asarray(inputs['bias1'], dtype=np.float32)
    W1full = np.concatenate([W1, r1[None]], axis=0)   # [9,2048,2048]
    W2full = np.concatenate([W2, r2[None]], axis=0)   # [9,2048,2048]

    cf = np.zeros((N_NODES, CONSTF_W), np.float32)
    cf[:, OFF_AT:OFF_AT + N_REL * N_NODES] = at
    cf[:, OFF_ID:OFF_ID + N_NODES] = np.eye(N_NODES)
    cf[0, OFF_ONES:OFF_ONES + N_NODES] = 1.0

    in_maps = []
    for c in range(N_CORES):
        cols = slice(c * CH, (c + 1) * CH)
        w1c = (W1full[:, :, cols]
               .reshape(9, KT, JT, CH))               # [9,128,16,256] f32
        h1c, l1c = _hilo(w1c)
        w1hl = (np.stack([h1c, l1c], axis=3)          # [9,128,16,2,256]
                .reshape(9, KT, JT * 2 * CH)).copy()
        w2c = (W2full[:, cols, :]
               .reshape(9, 2, KT, D)
               .transpose(0, 2, 1, 3))                # [9,128,2,2048] f32
        h2c, l2c = _hilo(w2c)
        w2hl = (np.stack([h2c, l2c], axis=3)          # [9,128,2,2,2048]
                .reshape(9, KT, 4 * D)).copy()
        cfc = cf.copy()
        cfc[0, OFF_B1:OFF_B1 + CH] = bias1[cols]
        in_maps.append({
            'w1': w1hl,
            'w2': w2hl,
            'xhl': xhl,
            'cf': cfc,
        })
    return in_maps


def get_compiled():
    global _compiled
    if _compiled is None:
        _compiled = _build()
    return _compiled


def run(inputs, trace=False):
    nc = get_compiled()
    in_maps = _prep_inputs(inputs)
    res = bass_utils.run_bass_kernel_spmd(
        nc, in_maps, core_ids=list(range(N_CORES)), trace=trace)
    acc = np.zeros((N_NODES, D), np.float64)
    for c in range(N_CORES):
        # out[p, q*17+m] = P_c[m, q*128+p] -> untranspose
        o = np.asarray(res.results[c]['out'], dtype=np.float64)
        acc += o.reshape(KT, JT, N_NODES).transpose(2, 1, 0).reshape(N_NODES, D)
    acc += np.asarray(inputs['bias2'], dtype=np.float64)[None, :]
    return acc.astype(np.float32), res


def kernel(**inputs):
    outp, _ = run(inputs, trace=False)
    return outp
